# revision 12
# baseline (speedup 1.0000x reference)
"""BilinearMixture kernel v2: u-sorted dedup.

Edges are globally sorted by u_idx and sharded 250k/core. Each core's
unique u-rows (~12.5k) are packed by the host into per-core slab windows of
128 rows; the kernel streams these sequentially (no u-side gathers). Each
128-edge tile expands its u-rows from the resident window with a one-hot
matmul (Sel, host-shipped fp16). Only the v-side uses per-tile indirect
gathers (the SWDGE descriptor-rate bound).

Static layout per core: G windows x T_CAP tiles x 128 edges.
"""

import sys

sys.path.insert(0, "/opt/trn_rl_repo")

import numpy as np
from contextlib import ExitStack

import concourse.bacc as bacc
import concourse.bass as bass
import concourse.mybir as mybir
import concourse.tile as tile
from concourse.bass_utils import run_bass_kernel_spmd
from concourse.masks import make_identity

F32 = mybir.dt.float32
F16 = mybir.dt.float16
I32 = mybir.dt.int32

NUM_USERS = 100000
NUM_ITEMS = 100000
D = 128
E = 2000000
NCLS = 5
N_CORES = 8

DEXT = 144
OUT_PAD = 16
T_CAP = 21          # tiles per window (2688 edge slots)
G_WIN = 103         # windows per core
E_SLOTS = G_WIN * T_CAP * 128   # 284416 slots per core
E_CORE = E // N_CORES


def build_v2_nc(n_rows_v, g_win=G_WIN, t_cap=T_CAP, d_ext=DEXT, ncls=NCLS,
                out_pad=OUT_PAD):
    nc = bacc.Bacc("TRN2", target_bir_lowering=False, debug=False)
    n_tiles = g_win * t_cap
    t_v = nc.dram_tensor("t_v", [n_rows_v, d_ext], F32, kind="ExternalInput").ap()
    # per-core packed unique-u slab rows (window g = rows 128g..128g+128)
    t_us = nc.dram_tensor("t_us", [g_win * 128, d_ext], F32,
                          kind="ExternalInput").ap()
    vix = nc.dram_tensor("vix", [g_win * 128, t_cap], I32,
                         kind="ExternalInput").ap()
    sel = nc.dram_tensor("sel", [n_tiles * 128, 128], F16,
                         kind="ExternalInput").ap()
    m0 = nc.dram_tensor("m0", [128, ncls], F32, kind="ExternalInput").ap()
    out = nc.dram_tensor("out", [n_tiles * 128, out_pad], F32,
                         kind="ExternalOutput").ap()

    with tile.TileContext(nc) as tc, ExitStack() as ctx:
        const_pool = ctx.enter_context(tc.tile_pool(name="const", bufs=1))
        slab_pool = ctx.enter_context(tc.tile_pool(name="slab", bufs=3))
        idx_pool = ctx.enter_context(tc.tile_pool(name="idx", bufs=4))
        sel_pool = ctx.enter_context(tc.tile_pool(name="sel", bufs=3))
        gv_pool = ctx.enter_context(tc.tile_pool(name="gv", bufs=8))
        prod_pool = ctx.enter_context(tc.tile_pool(name="prod", bufs=6))
        pts_pool = ctx.enter_context(tc.tile_pool(name="pts", bufs=4))
        osb_pool = ctx.enter_context(tc.tile_pool(name="osb", bufs=2))
        ux_psum = ctx.enter_context(tc.tile_pool(name="uxps", bufs=4,
                                                 space="PSUM"))
        ptp_pool = ctx.enter_context(tc.tile_pool(name="ptp", bufs=2,
                                                  space="PSUM"))
        ops_pool = ctx.enter_context(tc.tile_pool(name="ops", bufs=2,
                                                  space="PSUM"))

        identity = const_pool.tile([128, 128], F32)
        make_identity(nc, identity[:])
        m0_sb = const_pool.tile([128, ncls], F32)
        nc.sync.dma_start(out=m0_sb[:], in_=m0)

        out_sbs = [osb_pool.tile([128, t_cap * out_pad], F32, tag="osb",
                                 name=f"osb{i}") for i in range(2)]
        for t in out_sbs:
            nc.vector.memset(t[:], 0.0)

        for g in range(g_win):
            u_slab = slab_pool.tile([128, d_ext], F32, tag="us")
            nc.sync.dma_start(out=u_slab[:], in_=t_us[128 * g:128 * (g + 1), :])
            u_slab16 = slab_pool.tile([128, d_ext], F16, tag="us16")
            nc.vector.tensor_copy(out=u_slab16[:], in_=u_slab[:])

            vix_t = idx_pool.tile([128, t_cap], I32, tag="vix")
            nc.sync.dma_start(out=vix_t[:], in_=vix[128 * g:128 * (g + 1), :])

            # one batched Sel load per window instead of 22 tiny DMAs
            sel_blk = sel_pool.tile([128, t_cap * 128], F16, tag="sel")
            sel_src = sel[g * t_cap * 128:(g + 1) * t_cap * 128, :].rearrange(
                "(t p) q -> p t q", p=128)
            nc.sync.dma_start(
                out=sel_blk[:].rearrange("p (t q) -> p t q", q=128),
                in_=sel_src)

            ops = ops_pool.tile([128, t_cap * ncls], F32)
            out_sb = out_sbs[g % 2]
            osb3 = out_sb[:].rearrange("p (k c) -> p k c", c=out_pad)
            ops3 = ops[:].rearrange("p (k c) -> p k c", c=ncls)

            tile_base = g * t_cap
            LOOK = 3  # SW pipeline depth: expands run ahead of the
                      # transpose/reduce chain so PE never idles in-order
            vbl, uxl = {}, {}

            def tail(s):
                prod = prod_pool.tile([128, d_ext], F32, tag="p",
                                      name=f"prod_{g}_{s}")
                nc.vector.tensor_mul(out=prod[:], in0=uxl[s][:],
                                     in1=vbl[s][:])
                ptp = ptp_pool.tile([128, 128], F32, tag="tp",
                                    name=f"ptp_{g}_{s}")
                nc.tensor.transpose(out=ptp[:], in_=prod[:, 0:128],
                                    identity=identity[:])
                pts = pts_pool.tile([128, 128], F32, tag="pt",
                                    name=f"pts_{g}_{s}")
                nc.scalar.copy(out=pts[:], in_=ptp[:])
                nc.tensor.matmul(ops[:, ncls * s:ncls * (s + 1)], pts[:],
                                 m0_sb[:], start=True, stop=True)
                # out = feats@M0 + u_bias + v_bias
                nc.vector.tensor_add(
                    out=osb3[:, s, 0:ncls], in0=ops3[:, s, :],
                    in1=prod[:, 128:128 + ncls])
                nc.vector.tensor_add(
                    out=osb3[:, s, 0:ncls], in0=osb3[:, s, 0:ncls],
                    in1=prod[:, 133:133 + ncls])

            for t in range(t_cap):
                v_blk = gv_pool.tile([128, d_ext], F32, tag="v",
                                     name=f"vb_{g}_{t}")
                nc.gpsimd.indirect_dma_start(
                    out=v_blk[:], out_offset=None, in_=t_v,
                    in_offset=bass.IndirectOffsetOnAxis(
                        ap=vix_t[:, t:t + 1], axis=0))
                vbl[t] = v_blk

                # expand u rows: [e, d] = Sel^T.T @ slab
                ux = ux_psum.tile([128, d_ext], F32, tag="ux",
                                  name=f"ux_{g}_{t}")
                nc.tensor.matmul(ux[:], sel_blk[:, 128 * t:128 * (t + 1)],
                                 u_slab16[:], start=True, stop=True)
                uxl[t] = ux
                if t >= LOOK:
                    tail(t - LOOK)
            for s in range(max(0, t_cap - LOOK), t_cap):
                tail(s)

            row = 128 * t_cap * g
            dram3 = out[row:row + 128 * t_cap, :].rearrange(
                "(k p) c -> p k c", p=128)
            # store via ACT's HWDGE FIFO so next window's loads (slab/vix/
            # sel on SP's FIFO) are not serialized behind this store
            nc.scalar.dma_start(out=dram3, in_=osb3[:, :, :])

    nc.compile()
    return nc


def _pack_core(us, vs, row_tab, g_win, t_cap):
    """Pack one core's u-sorted edges into windows. Returns
    (t_us, vix, sel, slot_rows)."""
    n = len(us)
    n_tiles = g_win * t_cap
    e_cap = t_cap * 128
    uniq, first = np.unique(us, return_index=True)
    bounds = np.append(first, n)
    # greedy window assignment over unique users
    t_us = np.zeros((g_win * 128, row_tab.shape[1]), dtype=np.float32)
    vix = np.zeros((g_win * 128, t_cap), dtype=np.int32)
    sel_rows = np.empty(n, dtype=np.int64)
    sel_cols = np.empty(n, dtype=np.int64)
    slot_rows = np.empty(n, dtype=np.int64)
    g = 0
    i = 0          # unique-user cursor
    while i < len(uniq):
        estart = bounds[i]
        j = i
        while (j < len(uniq) and j - i < 128
               and bounds[j + 1] - estart <= e_cap):
            j += 1
        assert j > i, "single user exceeds window edge capacity"
        eend = bounds[j]
        m = j - i
        assert g < g_win, "ran out of windows; raise G_WIN"
        t_us[128 * g:128 * g + m] = row_tab[uniq[i:j]]
        # per-edge local user index within window
        loc = np.searchsorted(uniq[i:j], us[estart:eend])
        jj = np.arange(eend - estart)
        tt = jj // 128
        pp = jj % 128
        tile_idx = g * t_cap + tt
        vix[128 * g + pp, tt] = vs[estart:eend]
        sel_rows[estart:eend] = tile_idx * 128 + loc
        sel_cols[estart:eend] = pp
        slot_rows[estart:eend] = tile_idx * 128 + pp
        i = j
        g += 1
    sel = np.zeros((n_tiles * 128, 128), dtype=np.float16)
    sel[sel_rows, sel_cols] = 1.0
    return t_us, vix, sel, slot_rows


_NC2 = {}


def kernel(u_feats, v_feats, u_idx, v_idx, W, scalars, u_bias, v_bias,
           **run_kwargs):
    u_feats = np.asarray(u_feats, dtype=np.float32)
    v_feats = np.asarray(v_feats, dtype=np.float32)
    u_idx = np.asarray(u_idx, dtype=np.int32)
    v_idx = np.asarray(v_idx, dtype=np.int32)
    u_bias = np.asarray(u_bias, dtype=np.float32)
    v_bias = np.asarray(v_bias, dtype=np.float32)

    t_v = np.zeros((v_feats.shape[0], DEXT), dtype=np.float32)
    t_v[:, :D] = v_feats
    t_v[:, D:D + NCLS] = 1.0
    t_v[:, D + NCLS:D + 2 * NCLS] = v_bias
    row_tab = np.zeros((u_feats.shape[0], DEXT), dtype=np.float32)
    row_tab[:, :D] = u_feats
    row_tab[:, D:D + NCLS] = u_bias
    row_tab[:, D + NCLS:D + 2 * NCLS] = 1.0
    m0 = (np.asarray(W, np.float64).T @ np.asarray(scalars, np.float64)
          ).astype(np.float32)

    order = np.argsort(u_idx, kind="stable")
    in_maps = []
    slot_maps = []
    for c in range(N_CORES):
        oc = order[c * E_CORE:(c + 1) * E_CORE]
        t_us, vix, sel, slot_rows = _pack_core(
            u_idx[oc], v_idx[oc], row_tab, G_WIN, T_CAP)
        in_maps.append({"t_v": t_v, "t_us": t_us, "vix": vix, "sel": sel,
                        "m0": m0})
        slot_maps.append((oc, slot_rows))

    if "nc" not in _NC2:
        _NC2["nc"] = build_v2_nc(NUM_ITEMS)
    res = run_bass_kernel_spmd(_NC2["nc"], in_maps,
                               core_ids=list(range(N_CORES)), **run_kwargs)
    out = np.empty((E, NCLS), dtype=np.float32)
    for c in range(N_CORES):
        oc, slot_rows = slot_maps[c]
        out[oc] = res.results[c]["out"][slot_rows, :NCLS]
    if run_kwargs:
        kernel.last_result = res
    return out


# revision 14
# speedup vs baseline: 1.0153x; 1.0153x over previous
"""BilinearMixture kernel v2: u-sorted dedup.

Edges are globally sorted by u_idx and sharded 250k/core. Each core's
unique u-rows (~12.5k) are packed by the host into per-core slab windows of
128 rows; the kernel streams these sequentially (no u-side gathers). Each
128-edge tile expands its u-rows from the resident window with a one-hot
matmul (Sel, host-shipped fp16). Only the v-side uses per-tile indirect
gathers (the SWDGE descriptor-rate bound).

Static layout per core: G windows x T_CAP tiles x 128 edges.
"""

import sys

sys.path.insert(0, "/opt/trn_rl_repo")

import numpy as np
from contextlib import ExitStack

import concourse.bacc as bacc
import concourse.bass as bass
import concourse.mybir as mybir
import concourse.tile as tile
from concourse.bass_utils import run_bass_kernel_spmd
from concourse.masks import make_identity

F32 = mybir.dt.float32
F16 = mybir.dt.float16
I32 = mybir.dt.int32

NUM_USERS = 100000
NUM_ITEMS = 100000
D = 128
E = 2000000
NCLS = 5
N_CORES = 8

DEXT = 144
OUT_PAD = 16
T_CAP = 21          # tiles per window (2688 edge slots)
G_WIN = 103         # windows per core
E_SLOTS = G_WIN * T_CAP * 128   # 284416 slots per core
E_CORE = E // N_CORES


def build_v2_nc(n_rows_v, g_win=G_WIN, t_cap=T_CAP, d_ext=DEXT, ncls=NCLS,
                out_pad=OUT_PAD):
    nc = bacc.Bacc("TRN2", target_bir_lowering=False, debug=False)
    n_tiles = g_win * t_cap
    t_v = nc.dram_tensor("t_v", [n_rows_v, d_ext], F32, kind="ExternalInput").ap()
    # per-core packed unique-u slab rows (window g = rows 128g..128g+128)
    t_us = nc.dram_tensor("t_us", [g_win * 128, d_ext], F32,
                          kind="ExternalInput").ap()
    vix = nc.dram_tensor("vix", [g_win * 128, t_cap], I32,
                         kind="ExternalInput").ap()
    sel = nc.dram_tensor("sel", [n_tiles * 128, 128], F16,
                         kind="ExternalInput").ap()
    m0 = nc.dram_tensor("m0", [128, ncls], F32, kind="ExternalInput").ap()
    out = nc.dram_tensor("out", [n_tiles * 128, out_pad], F32,
                         kind="ExternalOutput").ap()

    with tile.TileContext(nc) as tc, ExitStack() as ctx:
        const_pool = ctx.enter_context(tc.tile_pool(name="const", bufs=1))
        slab_pool = ctx.enter_context(tc.tile_pool(name="slab", bufs=3))
        idx_pool = ctx.enter_context(tc.tile_pool(name="idx", bufs=4))
        sel_pool = ctx.enter_context(tc.tile_pool(name="sel", bufs=2))
        gv_pool = ctx.enter_context(tc.tile_pool(name="gv", bufs=8))
        prod_pool = ctx.enter_context(tc.tile_pool(name="prod", bufs=6))
        pts_pool = ctx.enter_context(tc.tile_pool(name="pts", bufs=4))
        osb_pool = ctx.enter_context(tc.tile_pool(name="osb", bufs=2))
        ux_psum = ctx.enter_context(tc.tile_pool(name="uxps", bufs=4,
                                                 space="PSUM"))
        ptp_pool = ctx.enter_context(tc.tile_pool(name="ptp", bufs=2,
                                                  space="PSUM"))
        ops_pool = ctx.enter_context(tc.tile_pool(name="ops", bufs=2,
                                                  space="PSUM"))

        identity = const_pool.tile([128, 128], F32)
        make_identity(nc, identity[:])
        m0_sb = const_pool.tile([128, ncls], F32)
        nc.sync.dma_start(out=m0_sb[:], in_=m0)

        # all v-indices resident up front: removes every per-window
        # dependency from the Pool gather stream
        vix_all = const_pool.tile([128, g_win * t_cap], I32)
        nc.sync.dma_start(
            out=vix_all[:].rearrange("p (g t) -> p g t", t=t_cap),
            in_=vix[:, :].rearrange("(g p) t -> p g t", p=128))

        out_sbs = [osb_pool.tile([128, t_cap * out_pad], F32, tag="osb",
                                 name=f"osb{i}") for i in range(2)]
        for t in out_sbs:
            nc.vector.memset(t[:], 0.0)

        for g in range(g_win):
            u_slab = slab_pool.tile([128, d_ext], F32, tag="us")
            nc.sync.dma_start(out=u_slab[:], in_=t_us[128 * g:128 * (g + 1), :])
            u_slab16 = slab_pool.tile([128, d_ext], F16, tag="us16")
            nc.vector.tensor_copy(out=u_slab16[:], in_=u_slab[:])

            # one batched Sel load per window instead of 22 tiny DMAs
            sel_blk = sel_pool.tile([128, t_cap * 128], F16, tag="sel")
            sel_src = sel[g * t_cap * 128:(g + 1) * t_cap * 128, :].rearrange(
                "(t p) q -> p t q", p=128)
            nc.sync.dma_start(
                out=sel_blk[:].rearrange("p (t q) -> p t q", q=128),
                in_=sel_src)

            ops = ops_pool.tile([128, t_cap * ncls], F32)
            out_sb = out_sbs[g % 2]
            osb3 = out_sb[:].rearrange("p (k c) -> p k c", c=out_pad)
            ops3 = ops[:].rearrange("p (k c) -> p k c", c=ncls)

            tile_base = g * t_cap
            LOOK = 3  # SW pipeline depth: expands run ahead of the
                      # transpose/reduce chain so PE never idles in-order
            vbl, uxl = {}, {}

            def tail(s):
                prod = prod_pool.tile([128, d_ext], F32, tag="p",
                                      name=f"prod_{g}_{s}")
                nc.vector.tensor_mul(out=prod[:], in0=uxl[s][:],
                                     in1=vbl[s][:])
                ptp = ptp_pool.tile([128, 128], F32, tag="tp",
                                    name=f"ptp_{g}_{s}")
                nc.tensor.transpose(out=ptp[:], in_=prod[:, 0:128],
                                    identity=identity[:])
                pts = pts_pool.tile([128, 128], F32, tag="pt",
                                    name=f"pts_{g}_{s}")
                nc.scalar.copy(out=pts[:], in_=ptp[:])
                nc.tensor.matmul(ops[:, ncls * s:ncls * (s + 1)], pts[:],
                                 m0_sb[:], start=True, stop=True)
                # out = feats@M0 + u_bias + v_bias
                nc.vector.tensor_add(
                    out=osb3[:, s, 0:ncls], in0=ops3[:, s, :],
                    in1=prod[:, 128:128 + ncls])
                nc.vector.tensor_add(
                    out=osb3[:, s, 0:ncls], in0=osb3[:, s, 0:ncls],
                    in1=prod[:, 133:133 + ncls])

            for t in range(t_cap):
                v_blk = gv_pool.tile([128, d_ext], F32, tag="v",
                                     name=f"vb_{g}_{t}")
                nc.gpsimd.indirect_dma_start(
                    out=v_blk[:], out_offset=None, in_=t_v,
                    in_offset=bass.IndirectOffsetOnAxis(
                        ap=vix_all[:, g * t_cap + t:g * t_cap + t + 1],
                        axis=0))
                vbl[t] = v_blk

                # expand u rows: [e, d] = Sel^T.T @ slab
                ux = ux_psum.tile([128, d_ext], F32, tag="ux",
                                  name=f"ux_{g}_{t}")
                nc.tensor.matmul(ux[:], sel_blk[:, 128 * t:128 * (t + 1)],
                                 u_slab16[:], start=True, stop=True)
                uxl[t] = ux
                if t >= LOOK:
                    tail(t - LOOK)
            for s in range(max(0, t_cap - LOOK), t_cap):
                tail(s)

            row = 128 * t_cap * g
            dram3 = out[row:row + 128 * t_cap, :].rearrange(
                "(k p) c -> p k c", p=128)
            nc.sync.dma_start(out=dram3, in_=osb3[:, :, :])

    nc.compile()
    return nc


def _pack_core(us, vs, row_tab, g_win, t_cap):
    """Pack one core's u-sorted edges into windows. Returns
    (t_us, vix, sel, slot_rows)."""
    n = len(us)
    n_tiles = g_win * t_cap
    e_cap = t_cap * 128
    uniq, first = np.unique(us, return_index=True)
    bounds = np.append(first, n)
    # greedy window assignment over unique users
    t_us = np.zeros((g_win * 128, row_tab.shape[1]), dtype=np.float32)
    vix = np.zeros((g_win * 128, t_cap), dtype=np.int32)
    sel_rows = np.empty(n, dtype=np.int64)
    sel_cols = np.empty(n, dtype=np.int64)
    slot_rows = np.empty(n, dtype=np.int64)
    g = 0
    i = 0          # unique-user cursor
    while i < len(uniq):
        estart = bounds[i]
        j = i
        while (j < len(uniq) and j - i < 128
               and bounds[j + 1] - estart <= e_cap):
            j += 1
        assert j > i, "single user exceeds window edge capacity"
        eend = bounds[j]
        m = j - i
        assert g < g_win, "ran out of windows; raise G_WIN"
        t_us[128 * g:128 * g + m] = row_tab[uniq[i:j]]
        # per-edge local user index within window
        loc = np.searchsorted(uniq[i:j], us[estart:eend])
        jj = np.arange(eend - estart)
        tt = jj // 128
        pp = jj % 128
        tile_idx = g * t_cap + tt
        vix[128 * g + pp, tt] = vs[estart:eend]
        sel_rows[estart:eend] = tile_idx * 128 + loc
        sel_cols[estart:eend] = pp
        slot_rows[estart:eend] = tile_idx * 128 + pp
        i = j
        g += 1
    sel = np.zeros((n_tiles * 128, 128), dtype=np.float16)
    sel[sel_rows, sel_cols] = 1.0
    return t_us, vix, sel, slot_rows


_NC2 = {}


def kernel(u_feats, v_feats, u_idx, v_idx, W, scalars, u_bias, v_bias,
           **run_kwargs):
    u_feats = np.asarray(u_feats, dtype=np.float32)
    v_feats = np.asarray(v_feats, dtype=np.float32)
    u_idx = np.asarray(u_idx, dtype=np.int32)
    v_idx = np.asarray(v_idx, dtype=np.int32)
    u_bias = np.asarray(u_bias, dtype=np.float32)
    v_bias = np.asarray(v_bias, dtype=np.float32)

    t_v = np.zeros((v_feats.shape[0], DEXT), dtype=np.float32)
    t_v[:, :D] = v_feats
    t_v[:, D:D + NCLS] = 1.0
    t_v[:, D + NCLS:D + 2 * NCLS] = v_bias
    row_tab = np.zeros((u_feats.shape[0], DEXT), dtype=np.float32)
    row_tab[:, :D] = u_feats
    row_tab[:, D:D + NCLS] = u_bias
    row_tab[:, D + NCLS:D + 2 * NCLS] = 1.0
    m0 = (np.asarray(W, np.float64).T @ np.asarray(scalars, np.float64)
          ).astype(np.float32)

    order = np.argsort(u_idx, kind="stable")
    in_maps = []
    slot_maps = []
    for c in range(N_CORES):
        oc = order[c * E_CORE:(c + 1) * E_CORE]
        t_us, vix, sel, slot_rows = _pack_core(
            u_idx[oc], v_idx[oc], row_tab, G_WIN, T_CAP)
        in_maps.append({"t_v": t_v, "t_us": t_us, "vix": vix, "sel": sel,
                        "m0": m0})
        slot_maps.append((oc, slot_rows))

    if "nc" not in _NC2:
        _NC2["nc"] = build_v2_nc(NUM_ITEMS)
    res = run_bass_kernel_spmd(_NC2["nc"], in_maps,
                               core_ids=list(range(N_CORES)), **run_kwargs)
    out = np.empty((E, NCLS), dtype=np.float32)
    for c in range(N_CORES):
        oc, slot_rows = slot_maps[c]
        out[oc] = res.results[c]["out"][slot_rows, :NCLS]
    if run_kwargs:
        kernel.last_result = res
    return out
